# revision 1
# baseline (speedup 1.0000x reference)
"""De-emphasis IIR filter x[n] = 0.95*x[n-1] + e[n] over axis 1 of (64, 480000) fp32.

Strategy:
  - Pure data parallel across 8 cores: 8 rows per core.
  - Within a core, each row of 480000 is split into 16 segments of 30000 so the
    128 SBUF partitions are all busy (8 rows x 16 segments).
  - Each segment is prefixed with a W-element warm-up region (the tail of the
    previous segment, zeros for the first segment).  0.95^W underflows far
    below fp32 resolution, so after the warm-up the scan state is identical
    (to fp32 precision) to the true carried state and segments become
    independent (overlap-save).
  - On-chip, the recurrence runs on the Vector engine via tensor_tensor_scan:
      state = (coeff * state) + e  along the free axis, per partition.
    Chunked along the free axis with carry chaining through `initial`;
    the scan runs in place on the DMA-landed tile; the coefficient operand is
    a [128,1] tile broadcast along the free axis with a stride-0 AP.
  - Input DMAs issue on the sync-engine HWDGE ring, output DMAs on the
    scalar-engine ring, so a waiting store never blocks loads.
  - This toolchain's codegen accepts at most ONE sync wait per instruction;
    _split_multi_waits rewrites any multi-wait instruction into single-wait
    NoOps preceding it on the same engine queue.
"""

import numpy as np

COEFF = 0.95
ROWS = 64
N = 480000
N_CORES = 8
RPC = ROWS // N_CORES  # rows per core = 8
NSEG = 16  # segments per row -> RPC*NSEG = 128 partitions
SEG = N // NSEG  # 30000
W = 336  # warm-up prefix; 0.95^336 ~ 3.3e-8 -> worst-case ~5e-7 abs error at
# segment starts, an order below the ~5.7e-6 fp32 reordering noise
TOT = SEG + W  # 30336 per partition on device
# Chunk schedule along the free axis (sums to TOT).  The cost model favors
# uniform fine chunks: the input-DMA ring streams at line rate regardless of
# chunk count, while the kernel tail (last scan + last store) shrinks with
# the final chunk's size.  952*4B = 3.8KB contiguous per partition per DMA
# keeps descriptors well above the 512B efficiency floor.  (Head/tail-trimmed
# and preload-last schedules were swept and model worse: the ring streams at
# line rate regardless, and extra tail chunks add latency hops.)
SIZES = [948] * 32

_cached = {}


def _build_bass(split_waits=True, sizes=None, w=W):
    """sizes: per-chunk free-axis lengths (must sum to SEG + w).  Asymmetric
    schedules (small first chunk -> output-DMA chain starts early, big middle
    chunks -> few per-DMA fixed costs, small last chunk -> short tail) beat a
    uniform split."""
    import concourse.bass as bass
    import concourse.mybir as mybir
    from concourse.tile import TileContext

    tot = SEG + w
    if sizes is None:
        sizes = SIZES
    assert sum(sizes) == tot, (sum(sizes), tot)
    assert sizes[0] > w

    f32 = mybir.dt.float32
    nc = bass.Bass(trn_type="TRN2")
    x = nc.dram_tensor("x", [128, tot], f32, kind="ExternalInput")
    y = nc.dram_tensor("y", [128, SEG], f32, kind="ExternalOutput")

    fmax = max(sizes)
    with TileContext(nc) as tc:
        with (
            tc.tile_pool(name="coef", bufs=1) as coefp,
            tc.tile_pool(name="io", bufs=min(len(sizes), 16)) as iop,
        ):
            ctile = coefp.tile([128, 1], f32)
            nc.vector.memset(ctile[:], COEFF)
            cap = ctile[:]
            cbcast = bass.AP(cap.tensor, cap.offset, [[cap.ap[0][0], 128], [0, fmax]])
            prev = None
            prev_f = 0
            off = 0
            for k, f in enumerate(sizes):
                tile = iop.tile([128, fmax], f32)
                nc.sync.dma_start(out=tile[:, 0:f], in_=x[:, off : off + f])
                init = 0.0 if prev is None else prev[:, prev_f - 1 : prev_f]
                cb = cbcast if f == fmax else bass.AP(
                    cap.tensor, cap.offset, [[cap.ap[0][0], 128], [0, f]]
                )
                nc.vector.tensor_tensor_scan(
                    out=tile[:, 0:f],
                    data0=cb,
                    data1=tile[:, 0:f],
                    initial=init,
                    op0=mybir.AluOpType.mult,
                    op1=mybir.AluOpType.add,
                )
                if k == 0:
                    nc.scalar.dma_start(out=y[:, 0 : f - w], in_=tile[:, w:f])
                else:
                    nc.scalar.dma_start(
                        out=y[:, off - w : off + f - w], in_=tile[:, 0:f]
                    )
                prev = tile
                prev_f = f
                off += f

    if split_waits:
        _split_multi_waits(nc, mybir)
    return nc


def _split_multi_waits(nc, mybir):
    """This walrus build rejects instructions carrying more than one sync
    wait (setupSyncWait: "Too many sync wait commands").  Split any
    multi-wait instruction into single-wait NoOps preceding it on the same
    engine queue (a wait executed earlier in queue order blocks identically)."""
    for fn in nc.m.functions:
        for blk in fn.blocks:
            out = []
            changed = False
            for inst in blk.instructions:
                si = inst.sync_info
                if si is not None and len(si.on_wait) > 1:
                    waits = list(si.on_wait)
                    for j, w_ in enumerate(waits[:-1]):
                        out.append(
                            mybir.InstNoOp(
                                name=f"splitwait-{inst.name}-{j}",
                                opcode="NoOp",
                                engine=inst.engine,
                                sync_info=mybir.SyncInfo(on_wait=[w_], on_update=[]),
                            )
                        )
                    si.on_wait = [waits[-1]]
                    inst.sync_info = si
                    changed = True
                out.append(inst)
            if changed:
                blk.instructions = out


def _shard_inputs(X, w=W):
    """X: (64, 480000) fp32 -> list of 8 per-core dicts {"x": (128, SEG+w)}."""
    tot = SEG + w
    in_maps = []
    for c in range(N_CORES):
        rows = X[c * RPC : (c + 1) * RPC]  # (8, N)
        padded = np.concatenate(
            [np.zeros((RPC, w), np.float32), rows], axis=1
        )  # (8, N+w)
        A = np.empty((RPC, NSEG, tot), np.float32)
        for s in range(NSEG):
            A[:, s, :] = padded[:, s * SEG : s * SEG + tot]
        in_maps.append({"x": np.ascontiguousarray(A.reshape(128, tot))})
    return in_maps


def _gather_outputs(results):
    out = np.empty((ROWS, N), dtype=np.float32)
    for c in range(N_CORES):
        O = results[c]["y"]  # (128, SEG)
        out[c * RPC : (c + 1) * RPC] = O.reshape(RPC, NSEG * SEG)
    return out


def run(X, trace=False):
    """Run on hardware; returns (output, BassKernelResults)."""
    from concourse.bass_utils import run_bass_kernel_spmd

    if "nc" not in _cached:
        _cached["nc"] = _build_bass()
    nc = _cached["nc"]
    in_maps = _shard_inputs(np.ascontiguousarray(X, dtype=np.float32))
    try:
        res = run_bass_kernel_spmd(
            nc, in_maps, core_ids=list(range(N_CORES)), trace=trace
        )
    except ModuleNotFoundError:
        # BASS_TRACE set but the axon NTFF hook (antenv.axon_hooks) is not
        # present in this container; run untraced instead of failing.
        import os

        os.environ["BASS_NEVER_TRACE"] = "1"
        res = run_bass_kernel_spmd(
            nc, in_maps, core_ids=list(range(N_CORES)), trace=False
        )
    return _gather_outputs(res.results), res


def kernel(inputs: np.ndarray) -> np.ndarray:
    out, _ = run(inputs, trace=False)
    return out

